# revision 5
# baseline (speedup 1.0000x reference)
"""Trainium2 Bass kernel for the NCA-style dense CNN problem.

Math (per batch image):
  y    = perception(x)  : 4 fixed 3x3 filters per channel, circular pad
  hid  = relu(w1 @ y + b1)   (1x1 conv 32->16)
  delta= w2 @ hid            (1x1 conv 16->8)
  out  = x + delta * mask

Host-side folding: perception + w1 collapse into a single 3x3 conv with
weights Weff[o,c,dy,dx] = sum_f w1[o,4c+f] * filt[f,dy,dx].

Device mapping (per core: 2 batch images, batch-sharded across 8 cores):
  - strip = 256 image rows; x tile [128 part = (16 blocks x 8 ch), 18 rows x 514]
    (1 halo row each side, 1 wrap col each side)
  - stage 1: 9 tap-shifted bf16 matmuls accumulate into PSUM.
    lhsT[(blk,c),(blk,hid)] block-diagonal [64,128]; two halves of the strip
    run on disjoint PE row-groups (partitions 0-63 / 64-127).
  - relu+bias on ACT (bias folded via per-partition bias AP), bf16 out
  - stage 2: block-diag [128,64] matmul (K=(blk,hid), M=(blk,c)),
    two halves packed into one PSUM bank via col-group tile_position.
  - mask: int32 -> bf16 cast, broadcast across the 8 channels with a tiny
    block-diag matmul into PSUM.
  - epilogue: DVE mul (delta*mask), GPSIMD add (+x), DMA out.
"""

import numpy as np
import ml_dtypes

B, C, H, W, HID = 16, 8, 512, 512, 16
NCORES = 8
BPC = B // NCORES          # batches per core
NBLK = 16                  # row-blocks per strip
RB = 16                    # rows per block
STRIP = NBLK * RB          # 256 rows
NSTRIP = H // STRIP        # 2 strips per image

_CACHE = {}


def _fixed_filters():
    ident = np.zeros((3, 3), np.float64)
    ident[1, 1] = 1.0
    sx = np.array([[-1.0, 0.0, 1.0], [-2.0, 0.0, 2.0], [-1.0, 0.0, 1.0]]) / 8.0
    lap = np.array([[1.0, 2.0, 1.0], [2.0, -12.0, 2.0], [1.0, 2.0, 1.0]]) / 16.0
    return np.stack([ident, sx, sx.T, lap])  # [4,3,3]


def _build_bass():
    import concourse.mybir as mybir
    from concourse import bacc, tile

    f32 = mybir.dt.float32
    bf16 = mybir.dt.bfloat16
    i32 = mybir.dt.int32
    Relu = mybir.ActivationFunctionType.Relu

    nc = bacc.Bacc(None, target_bir_lowering=False)
    x_d = nc.dram_tensor("x", (BPC, C, H, W), f32, kind="ExternalInput")
    m_d = nc.dram_tensor("update_mask", (BPC, 1, H, W), i32, kind="ExternalInput")
    w1t_d = nc.dram_tensor("w1t", (128, 9, 128), bf16, kind="ExternalInput")
    w2t_d = nc.dram_tensor("w2t", (128, 64), bf16, kind="ExternalInput")
    wmt_d = nc.dram_tensor("wmt", (128, 8, 128), bf16, kind="ExternalInput")
    b1_d = nc.dram_tensor("bias1", (128, 1), f32, kind="ExternalInput")
    o_d = nc.dram_tensor("out", (BPC, C, H, W), f32, kind="ExternalOutput")

    with tile.TileContext(nc) as tc:
        with (
            tc.tile_pool(name="consts", bufs=1) as cpool,
            tc.tile_pool(name="xf", bufs=2) as xfpool,
            tc.tile_pool(name="xb", bufs=2) as xbpool,
            tc.tile_pool(name="mk", bufs=2) as mkpool,
            tc.tile_pool(name="rl", bufs=3) as rlpool,
            tc.tile_pool(name="mf", bufs=3) as mfpool,
            tc.tile_pool(name="ost", bufs=2) as ostpool,
            tc.tile_pool(name="p1a", bufs=2, space="PSUM") as pp1a,
            tc.tile_pool(name="p1b", bufs=2, space="PSUM") as pp1b,
            tc.tile_pool(name="p2", bufs=2, space="PSUM") as pp2,
            tc.tile_pool(name="pm", bufs=2, space="PSUM") as ppm,
        ):
            w1t = cpool.tile([128, 9, 128], bf16)
            w2t = cpool.tile([128, 64], bf16)
            wmt = cpool.tile([128, 8, 128], bf16)
            b1t = cpool.tile([128, 1], f32)
            nc.sync.dma_start(out=w1t[:], in_=w1t_d[:])
            nc.sync.dma_start(out=w2t[:], in_=w2t_d[:])
            nc.sync.dma_start(out=wmt[:], in_=wmt_d[:])
            nc.sync.dma_start(out=b1t[:], in_=b1_d[:])

            for b in range(BPC):
                for s in range(NSTRIP):
                    r0 = s * STRIP
                    xt = xfpool.tile([128, RB + 2, W + 2], f32)
                    # core rows 16*blk .. +16 -> rows 1..17 of tile
                    # (split per channel: DMA AP balancer caps at 3 dims)
                    for c in range(C):
                        nc.sync.dma_start(
                            out=xt[c : 128 : C, 1 : RB + 1, 1 : W + 1],
                            in_=x_d[b, c, r0 : r0 + STRIP, :].rearrange(
                                "(blk r) w -> blk r w", blk=NBLK
                            ),
                        )
                    # halo-top rows: r0 + 16*blk - 1 (wrap at image top)
                    if r0 == 0:
                        nc.sync.dma_start(
                            out=xt[8:128, 0, 1 : W + 1],
                            in_=x_d[b, :, RB - 1 : STRIP - RB : RB, :].rearrange(
                                "c k w -> k c w"
                            ),
                        )
                        nc.sync.dma_start(
                            out=xt[0:8, 0, 1 : W + 1], in_=x_d[b, :, H - 1, :]
                        )
                    else:
                        nc.sync.dma_start(
                            out=xt[:, 0, 1 : W + 1],
                            in_=x_d[b, :, r0 - 1 : r0 + STRIP - RB : RB, :].rearrange(
                                "c k w -> k c w"
                            ),
                        )
                    # halo-bottom rows: r0 + 16*blk + 16 (wrap at image bottom)
                    if r0 + STRIP == H:
                        nc.sync.dma_start(
                            out=xt[0:120, RB + 1, 1 : W + 1],
                            in_=x_d[b, :, r0 + RB : H - RB + 1 : RB, :].rearrange(
                                "c k w -> k c w"
                            ),
                        )
                        nc.sync.dma_start(
                            out=xt[120:128, RB + 1, 1 : W + 1], in_=x_d[b, :, 0, :]
                        )
                    else:
                        nc.sync.dma_start(
                            out=xt[:, RB + 1, 1 : W + 1],
                            in_=x_d[b, :, r0 + RB : r0 + STRIP + 1 : RB, :].rearrange(
                                "c k w -> k c w"
                            ),
                        )
                    # wrap columns (free-dim copies, cheap)
                    nc.vector.tensor_copy(xt[:, :, 0], xt[:, :, W])
                    nc.vector.tensor_copy(xt[:, :, W + 1], xt[:, :, 1])
                    # bf16 cast of the whole haloed tile
                    xb = xbpool.tile([128, RB + 2, W + 2], bf16)
                    nc.vector.tensor_copy(xb[:], xt[:])
                    # mask: [128 part, 2 rows, 512]; partition p holds rows 2p,2p+1
                    mi = mkpool.tile([128, 2, W], i32, tag="mi")
                    nc.sync.dma_start(
                        out=mi[:],
                        in_=m_d[b, 0, r0 : r0 + STRIP, :].rearrange(
                            "(p r) w -> p r w", p=128
                        ),
                    )
                    mb = mkpool.tile([128, 2, W], bf16, tag="mb")
                    nc.vector.tensor_copy(mb[:], mi[:])

                    ov = o_d[b, :, r0 : r0 + STRIP, :].rearrange(
                        "c (blk half g) w -> half blk c (g w)", blk=NBLK, half=2, g=8
                    )
                    for half in range(2):
                        ost = ostpool.tile([128, 8, W], f32)
                        for gg in range(8):
                            g = half * 8 + gg
                            p1a = pp1a.tile([128, W], f32)
                            p1b = pp1b.tile([128, W], f32)
                            p2 = pp2.tile([128, W], f32)
                            pm = ppm.tile([128, W], f32)
                            # mask broadcast: psum[(blk,c), w] = mask[blk row g]
                            nc.tensor.matmul(
                                pm[:], wmt[:, g // 2, :], mb[:, g % 2, :],
                                start=True, stop=True,
                            )
                            # stage 1: 9 taps, strip-halves on disjoint row groups
                            for t in range(9):
                                i, j = t // 3, t % 3
                                nc.tensor.matmul(
                                    p1a[:],
                                    w1t[0:64, t, :],
                                    xb[0:64, g + i, j : j + W],
                                    start=(t == 0), stop=(t == 8),
                                )
                            for t in range(9):
                                i, j = t // 3, t % 3
                                nc.tensor.matmul(
                                    p1b[:],
                                    w1t[64:128, t, :],
                                    xb[64:128, g + i, j : j + W],
                                    start=(t == 0), stop=(t == 8),
                                )
                            # relu(+bias) -> bf16
                            ra = rlpool.tile([128, W], bf16, tag="ra")
                            rb = rlpool.tile([128, W], bf16, tag="rb")
                            nc.scalar.activation(ra[:], p1a[:], Relu, bias=b1t[:, 0:1])
                            nc.scalar.activation(rb[:], p1b[:], Relu, bias=b1t[:, 0:1])
                            # stage 2: both halves into one PSUM bank
                            nc.tensor.matmul(
                                p2[0:64, :], w2t[:], ra[:], start=True, stop=True
                            )
                            nc.tensor.matmul(
                                p2[64:128, :], w2t[:], rb[:],
                                start=True, stop=True, tile_position=(0, 64),
                            )
                            # epilogue: out = delta*mask + x
                            mf = mfpool.tile([128, W], f32)
                            nc.scalar.copy(mf[:], pm[:])
                            nc.vector.tensor_mul(ost[:, gg, :], p2[:], mf[:])
                            nc.gpsimd.tensor_add(
                                ost[:, gg, :], ost[:, gg, :], xt[:, g + 1, 1 : W + 1]
                            )
                        nc.sync.dma_start(out=ov[half], in_=ost[:])
    nc.compile()
    return nc


def _get_nc():
    if "nc" not in _CACHE:
        _CACHE["nc"] = _build_bass()
    return _CACHE["nc"]


def _fold_weights(w1_w, w1_b, w2_w):
    bf = ml_dtypes.bfloat16
    filt = _fixed_filters()  # [4,3,3] float64
    # Weff[o,c,i,j] = sum_f w1_w[o,4c+f] * filt[f,i,j]
    w1r = w1_w.astype(np.float64).reshape(HID, C, 4)
    weff = np.einsum("ocf,fij->ocij", w1r, filt)  # [16,8,3,3]

    w1t = np.zeros((128, 9, 128), np.float64)
    for blk in range(8):
        for c in range(C):
            for t in range(9):
                w1t[blk * 8 + c, t, blk * 16 : blk * 16 + 16] = weff[:, c, t // 3, t % 3]
    w1t[64:128] = w1t[0:64]

    w2t = np.zeros((128, 64), np.float64)
    for blk in range(8):
        for hid in range(HID):
            for co in range(C):
                w2t[blk * 16 + hid, blk * 8 + co] = w2_w[co, hid]

    wmt = np.zeros((128, 8, 128), np.float64)
    for blk in range(16):
        for sub in range(8):
            wmt[blk * 8 + sub, sub, blk * 8 : blk * 8 + 8] = 1.0

    b1 = np.zeros((128, 1), np.float32)
    for blk in range(8):
        b1[blk * 16 : blk * 16 + 16, 0] = w1_b

    return (
        np.ascontiguousarray(w1t.astype(bf)),
        np.ascontiguousarray(w2t.astype(bf)),
        np.ascontiguousarray(wmt.astype(bf)),
        b1,
    )


def kernel(x, w1_w, w1_b, w2_w, update_mask):
    from concourse.bass_utils import run_bass_kernel_spmd

    x = np.ascontiguousarray(np.asarray(x), dtype=np.float32)
    update_mask = np.ascontiguousarray(np.asarray(update_mask), dtype=np.int32)
    w1t, w2t, wmt, b1 = _fold_weights(
        np.asarray(w1_w, np.float64), np.asarray(w1_b, np.float64),
        np.asarray(w2_w, np.float64),
    )

    nc = _get_nc()
    in_maps = []
    for i in range(NCORES):
        in_maps.append(
            {
                "x": np.ascontiguousarray(x[i * BPC : (i + 1) * BPC]),
                "update_mask": np.ascontiguousarray(
                    update_mask[i * BPC : (i + 1) * BPC]
                ),
                "w1t": w1t,
                "w2t": w2t,
                "wmt": wmt,
                "bias1": b1,
            }
        )
    res = run_bass_kernel_spmd(nc, in_maps, core_ids=list(range(NCORES)))
    out = np.concatenate([r["out"] for r in res.results], axis=0)
    return out


# revision 12
# speedup vs baseline: 67.9532x; 67.9532x over previous
"""Trainium2 Bass kernel for the NCA-style dense CNN problem.

Math (per batch image):
  y    = perception(x)  : 4 fixed 3x3 filters per channel, circular pad
  hid  = relu(w1 @ y + b1)   (1x1 conv 32->16)
  delta= w2 @ hid            (1x1 conv 16->8)
  out  = x + delta * mask

Host-side folding: perception + w1 collapse into one 3x3 conv with weights
Weff[o,c,dy,dx] = sum_f w1[o,4c+f] * filt[f,dy,dx]. Each Weff row (fixed dy)
is then decomposed in the row-filter basis {delta=[0,1,0], s=[1,2,1],
d=[-1,0,1]}; for these filters the delta component is exactly zero at
dy=+-1, leaving 7 (source, dy) taps instead of 9 raw taps.

Device mapping (per core: 2 batch images, batch-sharded across 8 cores):
  - strip = 256 image rows; x tile [128 part = (16 blocks x 8 ch),
    18 rows x 514] bf16 (1 halo row each side, wrap cols), loaded with
    SWDGE cast-DMA straight from fp32 HBM.
  - DVE prefilter: H_s = x(w-1)+2x(w)+x(w+1), H_d = x(w+1)-x(w-1)
    (within-lane free-dim shifts).
  - stage 1: 7 tap matmuls (bf16) accumulate into PSUM; block-diagonal
    lhsT [64,128] = [(blk,c) -> (blk,hid)]; the two strip-halves run on
    disjoint PE row-groups (partitions 0-63 / 64-127).
  - relu+bias on ACT (per-partition bias AP), bf16 out
  - stage 2: block-diag [128,64] matmul, halves packed into one PSUM bank
    via col-group tile_position.
  - mask: int32 -> bf16 cast, broadcast across channels with a tiny
    block-diag matmul into PSUM.
  - epilogue: DVE mul (delta*mask), GPSIMD add (+x), DMA out.
"""

import numpy as np
import ml_dtypes

B, C, H, W, HID = 16, 8, 512, 512, 16
NCORES = 8
BPC = B // NCORES          # batches per core
NBLK = 16                  # row-blocks per strip
RB = 16                    # rows per block
STRIP = NBLK * RB          # 256 rows
NSTRIP = H // STRIP        # 2 strips per image

# taps: (source, dy) with source 0=x(delta row), 1=H_s, 2=H_d
TAPS = [(0, 0), (1, -1), (1, 0), (1, 1), (2, -1), (2, 0), (2, 1)]

_CACHE = {}


def _fixed_filters():
    ident = np.zeros((3, 3), np.float64)
    ident[1, 1] = 1.0
    sx = np.array([[-1.0, 0.0, 1.0], [-2.0, 0.0, 2.0], [-1.0, 0.0, 1.0]]) / 8.0
    lap = np.array([[1.0, 2.0, 1.0], [2.0, -12.0, 2.0], [1.0, 2.0, 1.0]]) / 16.0
    return np.stack([ident, sx, sx.T, lap])  # [4,3,3]


def _build_bass(ablate=()):
    import concourse.mybir as mybir
    from concourse import bacc, tile

    f32 = mybir.dt.float32
    bf16 = mybir.dt.bfloat16
    i32 = mybir.dt.int32
    Relu = mybir.ActivationFunctionType.Relu
    mul_op = mybir.AluOpType.mult
    add_op = mybir.AluOpType.add

    nc = bacc.Bacc(None, target_bir_lowering=False)
    x_d = nc.dram_tensor("x", (BPC, C, H, W), f32, kind="ExternalInput")
    m_d = nc.dram_tensor("update_mask", (BPC, 1, H, W), i32, kind="ExternalInput")
    w1t_d = nc.dram_tensor("w1t", (128, len(TAPS), 128), bf16, kind="ExternalInput")
    w2t_d = nc.dram_tensor("w2t", (128, 64), bf16, kind="ExternalInput")
    wmt_d = nc.dram_tensor("wmt", (128, 8, 128), bf16, kind="ExternalInput")
    b1_d = nc.dram_tensor("bias1", (128, 1), f32, kind="ExternalInput")
    o_d = nc.dram_tensor("out", (BPC, C, H, W), f32, kind="ExternalOutput")

    with tile.TileContext(nc) as tc:
        with (
            tc.tile_pool(name="consts", bufs=1) as cpool,
            tc.tile_pool(name="xb", bufs=2) as xbpool,
            tc.tile_pool(name="hmap", bufs=2) as hpool,
            tc.tile_pool(name="mk", bufs=2) as mkpool,
            tc.tile_pool(name="rl", bufs=3) as rlpool,
            tc.tile_pool(name="mf", bufs=3) as mfpool,
            tc.tile_pool(name="ost", bufs=2) as ostpool,
            tc.tile_pool(name="p1a", bufs=2, space="PSUM") as pp1a,
            tc.tile_pool(name="p1b", bufs=2, space="PSUM") as pp1b,
            tc.tile_pool(name="p2", bufs=2, space="PSUM") as pp2,
            tc.tile_pool(name="pm", bufs=2, space="PSUM") as ppm,
        ):
            w1t = cpool.tile([128, len(TAPS), 128], bf16)
            w2t = cpool.tile([128, 64], bf16)
            wmt = cpool.tile([128, 8, 128], bf16)
            b1t = cpool.tile([128, 1], f32)
            nc.sync.dma_start(out=w1t[:], in_=w1t_d[:])
            nc.sync.dma_start(out=w2t[:], in_=w2t_d[:])
            nc.sync.dma_start(out=wmt[:], in_=wmt_d[:])
            nc.sync.dma_start(out=b1t[:], in_=b1_d[:])

            for b in range(BPC):
                for s in range(NSTRIP):
                    r0 = s * STRIP
                    xb = xbpool.tile([128, RB + 2, W + 2], bf16)
                    # core rows 16*blk .. +16 -> rows 1..17 of tile
                    # (split per channel: DMA AP balancer caps at 3 dims;
                    #  SWDGE path casts fp32 -> bf16 in flight)
                    for c in range(C):
                        nc.gpsimd.dma_start(
                            out=xb[c : 128 : C, 1 : RB + 1, 1 : W + 1],
                            in_=x_d[b, c, r0 : r0 + STRIP, :].rearrange(
                                "(blk r) w -> blk r w", blk=NBLK
                            ),
                        )
                    # halo-top rows: r0 + 16*blk - 1 (wrap at image top)
                    if r0 == 0:
                        nc.gpsimd.dma_start(
                            out=xb[8:128, 0, 1 : W + 1],
                            in_=x_d[b, :, RB - 1 : STRIP - RB : RB, :].rearrange(
                                "c k w -> k c w"
                            ),
                        )
                        nc.gpsimd.dma_start(
                            out=xb[0:8, 0, 1 : W + 1], in_=x_d[b, :, H - 1, :]
                        )
                    else:
                        nc.gpsimd.dma_start(
                            out=xb[:, 0, 1 : W + 1],
                            in_=x_d[b, :, r0 - 1 : r0 + STRIP - RB : RB, :].rearrange(
                                "c k w -> k c w"
                            ),
                        )
                    # halo-bottom rows: r0 + 16*blk + 16 (wrap at image bottom)
                    if r0 + STRIP == H:
                        nc.gpsimd.dma_start(
                            out=xb[0:120, RB + 1, 1 : W + 1],
                            in_=x_d[b, :, r0 + RB : H - RB + 1 : RB, :].rearrange(
                                "c k w -> k c w"
                            ),
                        )
                        nc.gpsimd.dma_start(
                            out=xb[120:128, RB + 1, 1 : W + 1], in_=x_d[b, :, 0, :]
                        )
                    else:
                        nc.gpsimd.dma_start(
                            out=xb[:, RB + 1, 1 : W + 1],
                            in_=x_d[b, :, r0 + RB : r0 + STRIP + 1 : RB, :].rearrange(
                                "c k w -> k c w"
                            ),
                        )
                    # wrap columns (free-dim copies, cheap)
                    nc.vector.tensor_copy(xb[:, :, 0], xb[:, :, W])
                    nc.vector.tensor_copy(xb[:, :, W + 1], xb[:, :, 1])
                    # horizontal prefilter maps (within-lane shifts):
                    #   hs[k] = x[k] + 2 x[k+1] + x[k+2]   (= s_row * x at w=k+1)
                    #   hd[k] = x[k+2] - x[k]              (= d_row * x at w=k+1)
                    hs = hpool.tile([128, RB + 2, W], bf16, tag="hs")
                    hd = hpool.tile([128, RB + 2, W], bf16, tag="hd")
                    nc.vector.tensor_add(hs[:], xb[:, :, 0:W], xb[:, :, 2 : W + 2])
                    nc.vector.scalar_tensor_tensor(
                        hs[:], xb[:, :, 1 : W + 1], 2.0, hs[:], mul_op, add_op
                    )
                    nc.vector.tensor_sub(hd[:], xb[:, :, 2 : W + 2], xb[:, :, 0:W])
                    srcs = (xb, hs, hd)
                    # mask: [128 part, 2 rows, 512]; partition p holds rows 2p,2p+1
                    mi = mkpool.tile([128, 2, W], i32, tag="mi")
                    nc.sync.dma_start(
                        out=mi[:],
                        in_=m_d[b, 0, r0 : r0 + STRIP, :].rearrange(
                            "(p r) w -> p r w", p=128
                        ),
                    )
                    mb = mkpool.tile([128, 2, W], bf16, tag="mb")
                    nc.vector.tensor_copy(mb[:], mi[:])

                    ov = o_d[b, :, r0 : r0 + STRIP, :].rearrange(
                        "c (blk half g) w -> half blk c (g w)", blk=NBLK, half=2, g=8
                    )
                    for half in range(2):
                        ost = ostpool.tile([128, 8, W], f32)
                        for gg in range(8):
                            g = half * 8 + gg
                            p1a = pp1a.tile([128, W], f32)
                            p1b = pp1b.tile([128, W], f32)
                            p2 = pp2.tile([128, W], f32)
                            pm = ppm.tile([128, W], f32)
                            # mask broadcast: psum[(blk,c), w] = mask[blk row g]
                            nc.tensor.matmul(
                                pm[:], wmt[:, g // 2, :], mb[:, g % 2, :],
                                start=True, stop=True,
                            )
                            # stage 1: 7 taps, strip-halves on disjoint row groups
                            taps = TAPS[:1] if "tap1" in ablate else TAPS
                            nt = len(taps)
                            for t, (src, dy) in enumerate(taps):
                                st = srcs[src]
                                ap_a = (
                                    st[0:64, g + 1 + dy, 1 : W + 1]
                                    if src == 0
                                    else st[0:64, g + 1 + dy, :]
                                )
                                nc.tensor.matmul(
                                    p1a[:], w1t[0:64, t, :], ap_a,
                                    start=(t == 0), stop=(t == nt - 1),
                                )
                            for t, (src, dy) in enumerate(taps):
                                st = srcs[src]
                                ap_b = (
                                    st[64:128, g + 1 + dy, 1 : W + 1]
                                    if src == 0
                                    else st[64:128, g + 1 + dy, :]
                                )
                                nc.tensor.matmul(
                                    p1b[:], w1t[64:128, t, :], ap_b,
                                    start=(t == 0), stop=(t == nt - 1),
                                )
                            # relu(+bias) -> bf16
                            ra = rlpool.tile([128, W], bf16, tag="ra")
                            rb = rlpool.tile([128, W], bf16, tag="rb")
                            nc.scalar.activation(ra[:], p1a[:], Relu, bias=b1t[:, 0:1])
                            nc.scalar.activation(rb[:], p1b[:], Relu, bias=b1t[:, 0:1])
                            # stage 2: both halves into one PSUM bank
                            nc.tensor.matmul(
                                p2[0:64, :], w2t[:], ra[:], start=True, stop=True
                            )
                            nc.tensor.matmul(
                                p2[64:128, :], w2t[:], rb[:],
                                start=True, stop=True, tile_position=(0, 64),
                            )
                            # epilogue: out = delta*mask + x
                            if "psummul" in ablate:
                                nc.vector.tensor_mul(ost[:, gg, :], p2[:], pm[:])
                            else:
                                mf = mfpool.tile([128, W], f32)
                                nc.scalar.copy(mf[:], pm[:])
                                nc.vector.tensor_mul(ost[:, gg, :], p2[:], mf[:])
                            if "noadd" not in ablate:
                                # alternate the +x adds between GPSIMD and DVE
                                # (Pool also runs SWDGE descriptor generation)
                                eng = nc.gpsimd if gg % 2 == 0 else nc.vector
                                eng.tensor_add(
                                    ost[:, gg, :], ost[:, gg, :],
                                    xb[:, g + 1, 1 : W + 1],
                                )
                        nc.sync.dma_start(out=ov[half], in_=ost[:])
    nc.compile()
    return nc


def _get_nc():
    if "nc" not in _CACHE:
        _CACHE["nc"] = _build_bass()
    return _CACHE["nc"]


def _fold_weights(w1_w, w1_b, w2_w):
    bf = ml_dtypes.bfloat16
    filt = _fixed_filters()  # [4,3,3] float64
    w1r = w1_w.astype(np.float64).reshape(HID, C, 4)
    weff = np.einsum("ocf,fij->ocij", w1r, filt)  # [16,8,3,3]

    # decompose each row (fixed dy) in the basis {delta, s, d}
    basis = np.array(
        [[0.0, 1.0, 0.0], [1.0, 2.0, 1.0], [-1.0, 0.0, 1.0]], np.float64
    )  # rows: delta, s, d
    # weff[o,c,dy,:] = A[o,c,dy,0]*delta + A[o,c,dy,1]*s + A[o,c,dy,2]*d
    A = np.einsum("ocdr,rk->ocdk", weff, np.linalg.inv(basis))
    # delta component must vanish off-center for these filters
    assert np.abs(A[:, :, [0, 2], 0]).max() < 1e-10, "7-tap structure violated"

    w1t = np.zeros((128, len(TAPS), 128), np.float64)
    for blk in range(8):
        for c in range(C):
            for t, (src, dy) in enumerate(TAPS):
                w1t[blk * 8 + c, t, blk * 16 : blk * 16 + 16] = A[:, c, dy + 1, src]
    w1t[64:128] = w1t[0:64]

    w2t = np.zeros((128, 64), np.float64)
    for blk in range(8):
        for hid in range(HID):
            for co in range(C):
                w2t[blk * 16 + hid, blk * 8 + co] = w2_w[co, hid]

    wmt = np.zeros((128, 8, 128), np.float64)
    for blk in range(16):
        for sub in range(8):
            wmt[blk * 8 + sub, sub, blk * 8 : blk * 8 + 8] = 1.0

    b1 = np.zeros((128, 1), np.float32)
    for blk in range(8):
        b1[blk * 16 : blk * 16 + 16, 0] = w1_b

    return (
        np.ascontiguousarray(w1t.astype(bf)),
        np.ascontiguousarray(w2t.astype(bf)),
        np.ascontiguousarray(wmt.astype(bf)),
        b1,
    )


def kernel(x, w1_w, w1_b, w2_w, update_mask):
    from concourse.bass_utils import run_bass_kernel_spmd

    x = np.ascontiguousarray(np.asarray(x), dtype=np.float32)
    update_mask = np.ascontiguousarray(np.asarray(update_mask), dtype=np.int32)
    w1t, w2t, wmt, b1 = _fold_weights(
        np.asarray(w1_w, np.float64), np.asarray(w1_b, np.float64),
        np.asarray(w2_w, np.float64),
    )

    nc = _get_nc()
    in_maps = []
    for i in range(NCORES):
        in_maps.append(
            {
                "x": np.ascontiguousarray(x[i * BPC : (i + 1) * BPC]),
                "update_mask": np.ascontiguousarray(
                    update_mask[i * BPC : (i + 1) * BPC]
                ),
                "w1t": w1t,
                "w2t": w2t,
                "wmt": wmt,
                "bias1": b1,
            }
        )
    res = run_bass_kernel_spmd(nc, in_maps, core_ids=list(range(NCORES)))
    out = np.concatenate([r["out"] for r in res.results], axis=0)
    return out


# revision 13
# speedup vs baseline: 69.9730x; 1.0297x over previous
"""Trainium2 Bass kernel for the NCA-style dense CNN problem.

Math (per batch image):
  y    = perception(x)  : 4 fixed 3x3 filters per channel, circular pad
  hid  = relu(w1 @ y + b1)   (1x1 conv 32->16)
  delta= w2 @ hid            (1x1 conv 16->8)
  out  = x + delta * mask

Host-side folding: perception + w1 collapse into one 3x3 conv with weights
Weff[o,c,dy,dx] = sum_f w1[o,4c+f] * filt[f,dy,dx]. Each Weff row (fixed dy)
is then decomposed in the row-filter basis {delta=[0,1,0], s=[1,2,1],
d=[-1,0,1]}; for these filters the delta component is exactly zero at
dy=+-1, leaving 7 (source, dy) taps instead of 9 raw taps.

Device mapping (per core: 2 batch images, batch-sharded across 8 cores):
  - strip = 256 image rows; x tile [128 part = (16 blocks x 8 ch),
    18 rows x 514] bf16 (1 halo row each side, wrap cols), loaded with
    SWDGE cast-DMA straight from fp32 HBM.
  - DVE prefilter: H_s = x(w-1)+2x(w)+x(w+1), H_d = x(w+1)-x(w-1)
    (within-lane free-dim shifts).
  - stage 1: 7 tap matmuls (bf16) accumulate into PSUM; block-diagonal
    lhsT [64,128] = [(blk,c) -> (blk,hid)]; the two strip-halves run on
    disjoint PE row-groups (partitions 0-63 / 64-127).
  - relu+bias on ACT (per-partition bias AP), bf16 out
  - stage 2: block-diag [128,64] matmul, halves packed into one PSUM bank
    via col-group tile_position.
  - mask: int32 -> bf16 cast, broadcast across channels with a tiny
    block-diag matmul into PSUM.
  - epilogue: DVE mul (delta*mask), GPSIMD add (+x), DMA out.
"""

import numpy as np
import ml_dtypes

B, C, H, W, HID = 16, 8, 512, 512, 16
NCORES = 8
BPC = B // NCORES          # batches per core
NBLK = 16                  # row-blocks per strip
RB = 16                    # rows per block
STRIP = NBLK * RB          # 256 rows
NSTRIP = H // STRIP        # 2 strips per image

# taps: (source, dy) with source 0=x(delta row), 1=H_s, 2=H_d
TAPS = [(0, 0), (1, -1), (1, 0), (1, 1), (2, -1), (2, 0), (2, 1)]

_CACHE = {}


def _fixed_filters():
    ident = np.zeros((3, 3), np.float64)
    ident[1, 1] = 1.0
    sx = np.array([[-1.0, 0.0, 1.0], [-2.0, 0.0, 2.0], [-1.0, 0.0, 1.0]]) / 8.0
    lap = np.array([[1.0, 2.0, 1.0], [2.0, -12.0, 2.0], [1.0, 2.0, 1.0]]) / 16.0
    return np.stack([ident, sx, sx.T, lap])  # [4,3,3]


def _build_bass(ablate=()):
    import concourse.mybir as mybir
    from concourse import bacc, tile

    f32 = mybir.dt.float32
    bf16 = mybir.dt.bfloat16
    i32 = mybir.dt.int32
    Relu = mybir.ActivationFunctionType.Relu
    mul_op = mybir.AluOpType.mult
    add_op = mybir.AluOpType.add

    nc = bacc.Bacc(None, target_bir_lowering=False)
    x_d = nc.dram_tensor("x", (BPC, C, H, W), f32, kind="ExternalInput")
    m_d = nc.dram_tensor("update_mask", (BPC, 1, H, W), i32, kind="ExternalInput")
    w1t_d = nc.dram_tensor("w1t", (128, len(TAPS), 128), bf16, kind="ExternalInput")
    w2t_d = nc.dram_tensor("w2t", (128, 64), bf16, kind="ExternalInput")
    wmt_d = nc.dram_tensor("wmt", (128, 8, 128), bf16, kind="ExternalInput")
    b1_d = nc.dram_tensor("bias1", (128, 1), f32, kind="ExternalInput")
    o_d = nc.dram_tensor("out", (BPC, C, H, W), f32, kind="ExternalOutput")

    with tile.TileContext(nc) as tc:
        with (
            tc.tile_pool(name="consts", bufs=1) as cpool,
            tc.tile_pool(name="xb", bufs=2) as xbpool,
            tc.tile_pool(name="hmap", bufs=2) as hpool,
            tc.tile_pool(name="mk", bufs=2) as mkpool,
            tc.tile_pool(name="rl", bufs=3) as rlpool,
            tc.tile_pool(name="mf", bufs=3) as mfpool,
            tc.tile_pool(name="ost", bufs=2) as ostpool,
            tc.tile_pool(name="p1a", bufs=2, space="PSUM") as pp1a,
            tc.tile_pool(name="p1b", bufs=2, space="PSUM") as pp1b,
            tc.tile_pool(name="p2", bufs=2, space="PSUM") as pp2,
            tc.tile_pool(name="pm", bufs=2, space="PSUM") as ppm,
        ):
            w1t = cpool.tile([128, len(TAPS), 128], bf16)
            w2t = cpool.tile([128, 64], bf16)
            wmt = cpool.tile([128, 8, 128], bf16)
            b1t = cpool.tile([128, 1], f32)
            nc.sync.dma_start(out=w1t[:], in_=w1t_d[:])
            nc.sync.dma_start(out=w2t[:], in_=w2t_d[:])
            nc.sync.dma_start(out=wmt[:], in_=wmt_d[:])
            nc.sync.dma_start(out=b1t[:], in_=b1_d[:])

            for b in range(BPC):
                for s in range(NSTRIP):
                    r0 = s * STRIP
                    xb = xbpool.tile([128, RB + 2, W + 2], bf16)
                    # core rows 16*blk .. +16 -> rows 1..17 of tile
                    # (split per channel: DMA AP balancer caps at 3 dims;
                    #  SWDGE path casts fp32 -> bf16 in flight)
                    for c in range(C):
                        nc.gpsimd.dma_start(
                            out=xb[c : 128 : C, 1 : RB + 1, 1 : W + 1],
                            in_=x_d[b, c, r0 : r0 + STRIP, :].rearrange(
                                "(blk r) w -> blk r w", blk=NBLK
                            ),
                        )
                    # halo-top rows: r0 + 16*blk - 1 (wrap at image top)
                    if r0 == 0:
                        nc.gpsimd.dma_start(
                            out=xb[8:128, 0, 1 : W + 1],
                            in_=x_d[b, :, RB - 1 : STRIP - RB : RB, :].rearrange(
                                "c k w -> k c w"
                            ),
                        )
                        nc.gpsimd.dma_start(
                            out=xb[0:8, 0, 1 : W + 1], in_=x_d[b, :, H - 1, :]
                        )
                    else:
                        nc.gpsimd.dma_start(
                            out=xb[:, 0, 1 : W + 1],
                            in_=x_d[b, :, r0 - 1 : r0 + STRIP - RB : RB, :].rearrange(
                                "c k w -> k c w"
                            ),
                        )
                    # halo-bottom rows: r0 + 16*blk + 16 (wrap at image bottom)
                    if r0 + STRIP == H:
                        nc.gpsimd.dma_start(
                            out=xb[0:120, RB + 1, 1 : W + 1],
                            in_=x_d[b, :, r0 + RB : H - RB + 1 : RB, :].rearrange(
                                "c k w -> k c w"
                            ),
                        )
                        nc.gpsimd.dma_start(
                            out=xb[120:128, RB + 1, 1 : W + 1], in_=x_d[b, :, 0, :]
                        )
                    else:
                        nc.gpsimd.dma_start(
                            out=xb[:, RB + 1, 1 : W + 1],
                            in_=x_d[b, :, r0 + RB : r0 + STRIP + 1 : RB, :].rearrange(
                                "c k w -> k c w"
                            ),
                        )
                    # wrap columns (free-dim copies, cheap)
                    nc.vector.tensor_copy(xb[:, :, 0], xb[:, :, W])
                    nc.vector.tensor_copy(xb[:, :, W + 1], xb[:, :, 1])
                    # horizontal prefilter maps (within-lane shifts):
                    #   hs[k] = x[k] + 2 x[k+1] + x[k+2]   (= s_row * x at w=k+1)
                    #   hd[k] = x[k+2] - x[k]              (= d_row * x at w=k+1)
                    hs = hpool.tile([128, RB + 2, W], bf16, tag="hs")
                    hd = hpool.tile([128, RB + 2, W], bf16, tag="hd")
                    # computed in two row-ranges per strip-half (rows 0..9 and
                    # 8..17) so stage-1 matmuls of half 0 can start before the
                    # whole strip's prefilter is done
                    if "hsplit" in ablate:
                        rngs = [slice(0, RB + 2)]
                    else:
                        rngs = [slice(0, RB // 2 + 2), slice(RB // 2, RB + 2)]
                    for rr in rngs:
                        nc.vector.tensor_add(
                            hs[:, rr, :], xb[:, rr, 0:W], xb[:, rr, 2 : W + 2]
                        )
                        nc.vector.scalar_tensor_tensor(
                            hs[:, rr, :], xb[:, rr, 1 : W + 1], 2.0, hs[:, rr, :],
                            mul_op, add_op,
                        )
                        nc.vector.tensor_sub(
                            hd[:, rr, :], xb[:, rr, 2 : W + 2], xb[:, rr, 0:W]
                        )
                    srcs = (xb, hs, hd)
                    # mask: [128 part, 2 rows, 512]; partition p holds rows 2p,2p+1
                    mi = mkpool.tile([128, 2, W], i32, tag="mi")
                    nc.sync.dma_start(
                        out=mi[:],
                        in_=m_d[b, 0, r0 : r0 + STRIP, :].rearrange(
                            "(p r) w -> p r w", p=128
                        ),
                    )
                    mb = mkpool.tile([128, 2, W], bf16, tag="mb")
                    nc.vector.tensor_copy(mb[:], mi[:])

                    ov = o_d[b, :, r0 : r0 + STRIP, :].rearrange(
                        "c (blk half g) w -> half blk c (g w)", blk=NBLK, half=2, g=8
                    )
                    for half in range(2):
                        ost = ostpool.tile([128, 8, W], f32)
                        for gg in range(8):
                            g = half * 8 + gg
                            p1a = pp1a.tile([128, W], f32)
                            p1b = pp1b.tile([128, W], f32)
                            p2 = pp2.tile([128, W], f32)
                            pm = ppm.tile([128, W], f32)
                            # mask broadcast: psum[(blk,c), w] = mask[blk row g]
                            nc.tensor.matmul(
                                pm[:], wmt[:, g // 2, :], mb[:, g % 2, :],
                                start=True, stop=True,
                            )
                            # stage 1: 7 taps, strip-halves on disjoint row groups
                            taps = TAPS[:1] if "tap1" in ablate else TAPS
                            nt = len(taps)
                            for t, (src, dy) in enumerate(taps):
                                st = srcs[src]
                                ap_a = (
                                    st[0:64, g + 1 + dy, 1 : W + 1]
                                    if src == 0
                                    else st[0:64, g + 1 + dy, :]
                                )
                                nc.tensor.matmul(
                                    p1a[:], w1t[0:64, t, :], ap_a,
                                    start=(t == 0), stop=(t == nt - 1),
                                )
                            for t, (src, dy) in enumerate(taps):
                                st = srcs[src]
                                ap_b = (
                                    st[64:128, g + 1 + dy, 1 : W + 1]
                                    if src == 0
                                    else st[64:128, g + 1 + dy, :]
                                )
                                nc.tensor.matmul(
                                    p1b[:], w1t[64:128, t, :], ap_b,
                                    start=(t == 0), stop=(t == nt - 1),
                                )
                            # relu(+bias) -> bf16
                            ra = rlpool.tile([128, W], bf16, tag="ra")
                            rb = rlpool.tile([128, W], bf16, tag="rb")
                            nc.scalar.activation(ra[:], p1a[:], Relu, bias=b1t[:, 0:1])
                            nc.scalar.activation(rb[:], p1b[:], Relu, bias=b1t[:, 0:1])
                            # stage 2: both halves into one PSUM bank
                            nc.tensor.matmul(
                                p2[0:64, :], w2t[:], ra[:], start=True, stop=True
                            )
                            nc.tensor.matmul(
                                p2[64:128, :], w2t[:], rb[:],
                                start=True, stop=True, tile_position=(0, 64),
                            )
                            # epilogue: out = delta*mask + x
                            if "psummul" in ablate:
                                nc.vector.tensor_mul(ost[:, gg, :], p2[:], pm[:])
                            else:
                                mf = mfpool.tile([128, W], f32)
                                nc.scalar.copy(mf[:], pm[:])
                                nc.vector.tensor_mul(ost[:, gg, :], p2[:], mf[:])
                            if "noadd" not in ablate:
                                # alternate the +x adds between GPSIMD and DVE
                                # (Pool also runs SWDGE descriptor generation)
                                eng = nc.gpsimd if gg % 2 == 0 else nc.vector
                                eng.tensor_add(
                                    ost[:, gg, :], ost[:, gg, :],
                                    xb[:, g + 1, 1 : W + 1],
                                )
                        nc.sync.dma_start(out=ov[half], in_=ost[:])
    nc.compile()
    return nc


def _get_nc():
    if "nc" not in _CACHE:
        _CACHE["nc"] = _build_bass()
    return _CACHE["nc"]


def _fold_weights(w1_w, w1_b, w2_w):
    bf = ml_dtypes.bfloat16
    filt = _fixed_filters()  # [4,3,3] float64
    w1r = w1_w.astype(np.float64).reshape(HID, C, 4)
    weff = np.einsum("ocf,fij->ocij", w1r, filt)  # [16,8,3,3]

    # decompose each row (fixed dy) in the basis {delta, s, d}
    basis = np.array(
        [[0.0, 1.0, 0.0], [1.0, 2.0, 1.0], [-1.0, 0.0, 1.0]], np.float64
    )  # rows: delta, s, d
    # weff[o,c,dy,:] = A[o,c,dy,0]*delta + A[o,c,dy,1]*s + A[o,c,dy,2]*d
    A = np.einsum("ocdr,rk->ocdk", weff, np.linalg.inv(basis))
    # delta component must vanish off-center for these filters
    assert np.abs(A[:, :, [0, 2], 0]).max() < 1e-10, "7-tap structure violated"

    w1t = np.zeros((128, len(TAPS), 128), np.float64)
    for blk in range(8):
        for c in range(C):
            for t, (src, dy) in enumerate(TAPS):
                w1t[blk * 8 + c, t, blk * 16 : blk * 16 + 16] = A[:, c, dy + 1, src]
    w1t[64:128] = w1t[0:64]

    w2t = np.zeros((128, 64), np.float64)
    for blk in range(8):
        for hid in range(HID):
            for co in range(C):
                w2t[blk * 16 + hid, blk * 8 + co] = w2_w[co, hid]

    wmt = np.zeros((128, 8, 128), np.float64)
    for blk in range(16):
        for sub in range(8):
            wmt[blk * 8 + sub, sub, blk * 8 : blk * 8 + 8] = 1.0

    b1 = np.zeros((128, 1), np.float32)
    for blk in range(8):
        b1[blk * 16 : blk * 16 + 16, 0] = w1_b

    return (
        np.ascontiguousarray(w1t.astype(bf)),
        np.ascontiguousarray(w2t.astype(bf)),
        np.ascontiguousarray(wmt.astype(bf)),
        b1,
    )


def kernel(x, w1_w, w1_b, w2_w, update_mask):
    from concourse.bass_utils import run_bass_kernel_spmd

    x = np.ascontiguousarray(np.asarray(x), dtype=np.float32)
    update_mask = np.ascontiguousarray(np.asarray(update_mask), dtype=np.int32)
    w1t, w2t, wmt, b1 = _fold_weights(
        np.asarray(w1_w, np.float64), np.asarray(w1_b, np.float64),
        np.asarray(w2_w, np.float64),
    )

    nc = _get_nc()
    in_maps = []
    for i in range(NCORES):
        in_maps.append(
            {
                "x": np.ascontiguousarray(x[i * BPC : (i + 1) * BPC]),
                "update_mask": np.ascontiguousarray(
                    update_mask[i * BPC : (i + 1) * BPC]
                ),
                "w1t": w1t,
                "w2t": w2t,
                "wmt": wmt,
                "bias1": b1,
            }
        )
    res = run_bass_kernel_spmd(nc, in_maps, core_ids=list(range(NCORES)))
    out = np.concatenate([r["out"] for r in res.results], axis=0)
    return out


# revision 18
# speedup vs baseline: 70.3751x; 1.0057x over previous
"""Trainium2 Bass kernel for the NCA-style dense CNN problem.

Math (per batch image):
  y    = perception(x)  : 4 fixed 3x3 filters per channel, circular pad
  hid  = relu(w1 @ y + b1)   (1x1 conv 32->16)
  delta= w2 @ hid            (1x1 conv 16->8)
  out  = x + delta * mask

Host-side folding: perception + w1 collapse into one 3x3 conv with weights
Weff[o,c,dy,dx] = sum_f w1[o,4c+f] * filt[f,dy,dx]. Each Weff row (fixed dy)
is then decomposed in the row-filter basis {delta=[0,1,0], s=[1,2,1],
d=[-1,0,1]}; for these filters the delta component is exactly zero at
dy=+-1, leaving 7 (source, dy) taps instead of 9 raw taps.

Device mapping (per core: 2 batch images, batch-sharded across 8 cores):
  - strip = 256 image rows; x tile [128 part = (16 blocks x 8 ch),
    18 rows x 514] bf16 (1 halo row each side, wrap cols), loaded with
    SWDGE cast-DMA straight from fp32 HBM.
  - DVE prefilter: H_s = x(w-1)+2x(w)+x(w+1), H_d = x(w+1)-x(w-1)
    (within-lane free-dim shifts).
  - stage 1: 7 tap matmuls (bf16) accumulate into PSUM; block-diagonal
    lhsT [64,128] = [(blk,c) -> (blk,hid)]; the two strip-halves run on
    disjoint PE row-groups (partitions 0-63 / 64-127).
  - relu+bias on ACT (per-partition bias AP), bf16 out
  - stage 2: block-diag [128,64] matmul, halves packed into one PSUM bank
    via col-group tile_position.
  - mask: int32 -> bf16 cast, broadcast across channels with a tiny
    block-diag matmul into PSUM.
  - epilogue: DVE mul (delta*mask), GPSIMD add (+x), DMA out.
"""

import numpy as np
import ml_dtypes

B, C, H, W, HID = 16, 8, 512, 512, 16
NCORES = 8
BPC = B // NCORES          # batches per core
NBLK = 16                  # row-blocks per strip
RB = 16                    # rows per block
STRIP = NBLK * RB          # 256 rows
NSTRIP = H // STRIP        # 2 strips per image

# taps: (source, dy) with source 0=x(delta row), 1=H_s, 2=H_d
TAPS = [(0, 0), (1, -1), (1, 0), (1, 1), (2, -1), (2, 0), (2, 1)]

_CACHE = {}


def _fixed_filters():
    ident = np.zeros((3, 3), np.float64)
    ident[1, 1] = 1.0
    sx = np.array([[-1.0, 0.0, 1.0], [-2.0, 0.0, 2.0], [-1.0, 0.0, 1.0]]) / 8.0
    lap = np.array([[1.0, 2.0, 1.0], [2.0, -12.0, 2.0], [1.0, 2.0, 1.0]]) / 16.0
    return np.stack([ident, sx, sx.T, lap])  # [4,3,3]


def _build_bass(ablate=()):
    import concourse.mybir as mybir
    from concourse import bacc, tile

    f32 = mybir.dt.float32
    bf16 = mybir.dt.bfloat16
    i32 = mybir.dt.int32
    Relu = mybir.ActivationFunctionType.Relu
    mul_op = mybir.AluOpType.mult
    add_op = mybir.AluOpType.add

    nc = bacc.Bacc(None, target_bir_lowering=False)
    x_d = nc.dram_tensor("x", (BPC, C, H, W), f32, kind="ExternalInput")
    m_d = nc.dram_tensor("update_mask", (BPC, 1, H, W), i32, kind="ExternalInput")
    w1t_d = nc.dram_tensor("w1t", (128, len(TAPS), 128), bf16, kind="ExternalInput")
    w2t_d = nc.dram_tensor("w2t", (128, 64), bf16, kind="ExternalInput")
    wmt_d = nc.dram_tensor("wmt", (128, 8, 128), bf16, kind="ExternalInput")
    b1_d = nc.dram_tensor("bias1", (128, 1), f32, kind="ExternalInput")
    o_d = nc.dram_tensor("out", (BPC, C, H, W), f32, kind="ExternalOutput")

    with tile.TileContext(nc) as tc:
        with (
            tc.tile_pool(name="consts", bufs=1) as cpool,
            tc.tile_pool(name="xb", bufs=3) as xbpool,
            tc.tile_pool(name="hmap", bufs=2) as hpool,
            tc.tile_pool(name="mk", bufs=2) as mkpool,
            tc.tile_pool(name="rl", bufs=3) as rlpool,
            tc.tile_pool(name="mf", bufs=3) as mfpool,
            tc.tile_pool(name="ost", bufs=2) as ostpool,
            tc.tile_pool(name="p1a", bufs=2, space="PSUM") as pp1a,
            tc.tile_pool(name="p1b", bufs=2, space="PSUM") as pp1b,
            tc.tile_pool(name="p2", bufs=2, space="PSUM") as pp2,
            tc.tile_pool(name="pm", bufs=2, space="PSUM") as ppm,
        ):
            w1t = cpool.tile([128, len(TAPS), 128], bf16)
            w2t = cpool.tile([128, 64], bf16)
            wmt = cpool.tile([128, 8, 128], bf16)
            b1t = cpool.tile([128, 1], f32)
            nc.sync.dma_start(out=w1t[:], in_=w1t_d[:])
            nc.sync.dma_start(out=w2t[:], in_=w2t_d[:])
            nc.sync.dma_start(out=wmt[:], in_=wmt_d[:])
            nc.sync.dma_start(out=b1t[:], in_=b1_d[:])

            for b in range(BPC):
                for s in range(NSTRIP):
                    r0 = s * STRIP
                    xb = xbpool.tile([128, RB + 2, W + 2], bf16)
                    # core rows 16*blk .. +16 -> rows 1..17 of tile
                    # (split per channel: DMA AP balancer caps at 3 dims;
                    #  SWDGE path casts fp32 -> bf16 in flight)
                    for c in range(C):
                        nc.gpsimd.dma_start(
                            out=xb[c : 128 : C, 1 : RB + 1, 1 : W + 1],
                            in_=x_d[b, c, r0 : r0 + STRIP, :].rearrange(
                                "(blk r) w -> blk r w", blk=NBLK
                            ),
                        )
                    # halo-top rows: r0 + 16*blk - 1 (wrap at image top)
                    if r0 == 0:
                        nc.gpsimd.dma_start(
                            out=xb[8:128, 0, 1 : W + 1],
                            in_=x_d[b, :, RB - 1 : STRIP - RB : RB, :].rearrange(
                                "c k w -> k c w"
                            ),
                        )
                        nc.gpsimd.dma_start(
                            out=xb[0:8, 0, 1 : W + 1], in_=x_d[b, :, H - 1, :]
                        )
                    else:
                        nc.gpsimd.dma_start(
                            out=xb[:, 0, 1 : W + 1],
                            in_=x_d[b, :, r0 - 1 : r0 + STRIP - RB : RB, :].rearrange(
                                "c k w -> k c w"
                            ),
                        )
                    # halo-bottom rows: r0 + 16*blk + 16 (wrap at image bottom)
                    if r0 + STRIP == H:
                        nc.gpsimd.dma_start(
                            out=xb[0:120, RB + 1, 1 : W + 1],
                            in_=x_d[b, :, r0 + RB : H - RB + 1 : RB, :].rearrange(
                                "c k w -> k c w"
                            ),
                        )
                        nc.gpsimd.dma_start(
                            out=xb[120:128, RB + 1, 1 : W + 1], in_=x_d[b, :, 0, :]
                        )
                    else:
                        nc.gpsimd.dma_start(
                            out=xb[:, RB + 1, 1 : W + 1],
                            in_=x_d[b, :, r0 + RB : r0 + STRIP + 1 : RB, :].rearrange(
                                "c k w -> k c w"
                            ),
                        )
                    # wrap columns (free-dim copies, cheap)
                    nc.vector.tensor_copy(xb[:, :, 0], xb[:, :, W])
                    nc.vector.tensor_copy(xb[:, :, W + 1], xb[:, :, 1])
                    # horizontal prefilter maps (within-lane shifts):
                    #   hs[k] = x[k] + 2 x[k+1] + x[k+2]   (= s_row * x at w=k+1)
                    #   hd[k] = x[k+2] - x[k]              (= d_row * x at w=k+1)
                    hs = hpool.tile([128, RB + 2, W], bf16, tag="hs")
                    hd = hpool.tile([128, RB + 2, W], bf16, tag="hd")
                    # computed in two row-ranges per strip-half (rows 0..9 and
                    # 8..17) so stage-1 matmuls of half 0 can start before the
                    # whole strip's prefilter is done
                    if "hsplit" in ablate:
                        rngs = [slice(0, RB + 2)]
                    else:
                        rngs = [slice(0, RB // 2 + 2), slice(RB // 2, RB + 2)]
                    for rr in rngs:
                        nc.vector.tensor_add(
                            hs[:, rr, :], xb[:, rr, 0:W], xb[:, rr, 2 : W + 2]
                        )
                        nc.vector.scalar_tensor_tensor(
                            hs[:, rr, :], xb[:, rr, 1 : W + 1], 2.0, hs[:, rr, :],
                            mul_op, add_op,
                        )
                        nc.vector.tensor_sub(
                            hd[:, rr, :], xb[:, rr, 2 : W + 2], xb[:, rr, 0:W]
                        )
                    srcs = (xb, hs, hd)
                    # mask: [128 part, 2 rows, 512]; partition p holds rows 2p,2p+1
                    mi = mkpool.tile([128, 2, W], i32, tag="mi")
                    nc.sync.dma_start(
                        out=mi[:],
                        in_=m_d[b, 0, r0 : r0 + STRIP, :].rearrange(
                            "(p r) w -> p r w", p=128
                        ),
                    )
                    mb = mkpool.tile([128, 2, W], bf16, tag="mb")
                    nc.vector.tensor_copy(mb[:], mi[:])

                    ov = o_d[b, :, r0 : r0 + STRIP, :].rearrange(
                        "c (blk half g) w -> half blk c (g w)", blk=NBLK, half=2, g=8
                    )
                    for half in range(2):
                        ost = ostpool.tile([128, 8, W], f32)
                        for gg in range(8):
                            g = half * 8 + gg
                            p1a = pp1a.tile([128, W], f32)
                            p1b = pp1b.tile([128, W], f32)
                            p2 = pp2.tile([128, W], f32)
                            pm = ppm.tile([128, W], f32)
                            # mask broadcast: psum[(blk,c), w] = mask[blk row g]
                            nc.tensor.matmul(
                                pm[:], wmt[:, g // 2, :], mb[:, g % 2, :],
                                start=True, stop=True,
                            )
                            # stage 1: 7 taps, strip-halves on disjoint row groups
                            taps = TAPS[:1] if "tap1" in ablate else TAPS
                            nt = len(taps)
                            for t, (src, dy) in enumerate(taps):
                                st = srcs[src]
                                ap_a = (
                                    st[0:64, g + 1 + dy, 1 : W + 1]
                                    if src == 0
                                    else st[0:64, g + 1 + dy, :]
                                )
                                nc.tensor.matmul(
                                    p1a[:], w1t[0:64, t, :], ap_a,
                                    start=(t == 0), stop=(t == nt - 1),
                                )
                            for t, (src, dy) in enumerate(taps):
                                st = srcs[src]
                                ap_b = (
                                    st[64:128, g + 1 + dy, 1 : W + 1]
                                    if src == 0
                                    else st[64:128, g + 1 + dy, :]
                                )
                                nc.tensor.matmul(
                                    p1b[:], w1t[64:128, t, :], ap_b,
                                    start=(t == 0), stop=(t == nt - 1),
                                )
                            # relu(+bias) -> bf16
                            ra = rlpool.tile([128, W], bf16, tag="ra")
                            rb = rlpool.tile([128, W], bf16, tag="rb")
                            nc.scalar.activation(ra[:], p1a[:], Relu, bias=b1t[:, 0:1])
                            nc.scalar.activation(rb[:], p1b[:], Relu, bias=b1t[:, 0:1])
                            # stage 2: both halves into one PSUM bank
                            nc.tensor.matmul(
                                p2[0:64, :], w2t[:], ra[:], start=True, stop=True
                            )
                            nc.tensor.matmul(
                                p2[64:128, :], w2t[:], rb[:],
                                start=True, stop=True, tile_position=(0, 64),
                            )
                            # epilogue: out = delta*mask + x
                            if "psummul" in ablate:
                                nc.vector.tensor_mul(ost[:, gg, :], p2[:], pm[:])
                            else:
                                mf = mfpool.tile([128, W], f32)
                                nc.scalar.copy(mf[:], pm[:])
                                nc.vector.tensor_mul(ost[:, gg, :], p2[:], mf[:])
                            if "noadd" not in ablate:
                                # alternate the +x adds between GPSIMD and DVE
                                # (Pool also runs SWDGE descriptor generation)
                                eng = nc.gpsimd if gg % 2 == 0 else nc.vector
                                eng.tensor_add(
                                    ost[:, gg, :], ost[:, gg, :],
                                    xb[:, g + 1, 1 : W + 1],
                                )
                        nc.sync.dma_start(out=ov[half], in_=ost[:])
    nc.compile()
    return nc


def _get_nc():
    if "nc" not in _CACHE:
        _CACHE["nc"] = _build_bass()
    return _CACHE["nc"]


def _fold_weights(w1_w, w1_b, w2_w):
    bf = ml_dtypes.bfloat16
    filt = _fixed_filters()  # [4,3,3] float64
    w1r = w1_w.astype(np.float64).reshape(HID, C, 4)
    weff = np.einsum("ocf,fij->ocij", w1r, filt)  # [16,8,3,3]

    # decompose each row (fixed dy) in the basis {delta, s, d}
    basis = np.array(
        [[0.0, 1.0, 0.0], [1.0, 2.0, 1.0], [-1.0, 0.0, 1.0]], np.float64
    )  # rows: delta, s, d
    # weff[o,c,dy,:] = A[o,c,dy,0]*delta + A[o,c,dy,1]*s + A[o,c,dy,2]*d
    A = np.einsum("ocdr,rk->ocdk", weff, np.linalg.inv(basis))
    # delta component must vanish off-center for these filters
    assert np.abs(A[:, :, [0, 2], 0]).max() < 1e-10, "7-tap structure violated"

    w1t = np.zeros((128, len(TAPS), 128), np.float64)
    for blk in range(8):
        for c in range(C):
            for t, (src, dy) in enumerate(TAPS):
                w1t[blk * 8 + c, t, blk * 16 : blk * 16 + 16] = A[:, c, dy + 1, src]
    w1t[64:128] = w1t[0:64]

    w2t = np.zeros((128, 64), np.float64)
    for blk in range(8):
        for hid in range(HID):
            for co in range(C):
                w2t[blk * 16 + hid, blk * 8 + co] = w2_w[co, hid]

    wmt = np.zeros((128, 8, 128), np.float64)
    for blk in range(16):
        for sub in range(8):
            wmt[blk * 8 + sub, sub, blk * 8 : blk * 8 + 8] = 1.0

    b1 = np.zeros((128, 1), np.float32)
    for blk in range(8):
        b1[blk * 16 : blk * 16 + 16, 0] = w1_b

    return (
        np.ascontiguousarray(w1t.astype(bf)),
        np.ascontiguousarray(w2t.astype(bf)),
        np.ascontiguousarray(wmt.astype(bf)),
        b1,
    )


def kernel(x, w1_w, w1_b, w2_w, update_mask):
    from concourse.bass_utils import run_bass_kernel_spmd

    x = np.ascontiguousarray(np.asarray(x), dtype=np.float32)
    update_mask = np.ascontiguousarray(np.asarray(update_mask), dtype=np.int32)
    w1t, w2t, wmt, b1 = _fold_weights(
        np.asarray(w1_w, np.float64), np.asarray(w1_b, np.float64),
        np.asarray(w2_w, np.float64),
    )

    nc = _get_nc()
    in_maps = []
    for i in range(NCORES):
        in_maps.append(
            {
                "x": np.ascontiguousarray(x[i * BPC : (i + 1) * BPC]),
                "update_mask": np.ascontiguousarray(
                    update_mask[i * BPC : (i + 1) * BPC]
                ),
                "w1t": w1t,
                "w2t": w2t,
                "wmt": wmt,
                "bias1": b1,
            }
        )
    res = run_bass_kernel_spmd(nc, in_maps, core_ids=list(range(NCORES)))
    out = np.concatenate([r["out"] for r in res.results], axis=0)
    return out
